# revision 83
# baseline (speedup 1.0000x reference)
"""Bass/Trainium2 kernel for DynamicMultiheadAttention (sparse_attention).

Sharding: 8 cores = (batch b in {0,1}) x (query-slice of 512 rows).
Each core computes all 8 heads for its (b, n-slice) in transposed
orientation: scores sT[m, n] with keys m on partitions, so that
  - the relative-mask bias  -sum_r c[h,r]*M_r[m,n]  is accumulated into
    score PSUM by fp8 DoubleRow matmuls (0.5 cycles/row, two mask
    planes per instruction) with scaled-identity stationary operands.
    Masks are 0/1 (exact in fp8); each coefficient is split
    c = fp8(c) + fp8(c - fp8(c)).  Mask planes are staged as
    [M0,M1,M2,M0] so DoubleRow pairs (0,1),(1,2),(2,3) can cover each
    plane twice (main + residual) with stride-adjacent slices.  By
    default every head runs just two DoubleRows -- pairs (0,1) and
    (2,3), i.e. all mains plus the r0 residual -- which costs ~1.1e-2
    end-to-end vs the 2e-2 budget; KB_K3=n instead gives the n heads
    with the largest |residual|*exp(-c) impact the full three-DoubleRow
    (exact to ~4e-3) path.
  - softmax row-sums come free from a ones-column appended to V,
  - attn @ V needs no transposes (pT tiles are directly the stationary
    operand layout).
Key padding is applied by zeroing padded key rows of V and of the
ones-column (exactly equivalent to -inf logits). The row-constant term
scale_h * sum_r w[h,r] = scale_h cancels in softmax and is dropped; the
k-projection bias is softmax-invariant and dropped; the v bias folds
into the output bias (softmax rows sum to 1): bo' = bv @ Wo + bo.

The q/k/v projections run as split-fp8 DoubleRows: the host stages
16*W = w8 + wr and x = x8 + xr (two fp8e4 tensors each, same bytes as
bf16), the kernel accumulates w8x8 + w8xr + wrx8 over 256-deep k-tile
pairs (the wrxr term is ~0.1% and dropped) and the epilogue scales by
1/16 (1/128 for q, folding in the 1/sqrt(C) score scale) -- ~2.4x
fewer PE cycles than bf16 at bf16-level accuracy.  The 16x staging
keeps the weight residuals out of fp8's subnormal range.

Attention runs in 4 passes of 2 heads so PSUM fits: per (head,
mt-pair) a [128,2,512] score tile (2 banks) accumulates QK (bf16) plus
the mask DoubleRows, one Exp activation covers both tiles (halves the
Activation-engine instruction count), and two bf16 attn@V matmuls
drain it into the per-head output accumulator.  The v-projection is
emitted inside pass 0, one pair ahead of its attn@V consumer, so phase
B starts right after the k-projection.  Each pass's normalization is
emitted after the next pass's first pair so the DVE reciprocal latency
hides under PE work; the output projection pre-accumulates heads 0-5
before the last normalization and only the g=3 matmuls, split Act/DVE
epilogues, and dual-queue output DMAs sit in the drain tail.

DMA schedule is ordered by first use (xtq/wq -> wk/xtk -> masks/id8
interleaved with wv/xtv -> wo), with the 512-row tensors loaded in
full-height column blocks so each projection sub-block starts as soon
as its operands land.

Measured on the staged harness: rel err 1.11e-2, HW exec 131356 ns
(timeline-sim estimate; baseline was 200796 ns).

Every TPB instruction encoding in this walrus build tolerates only ONE
semaphore wait; a post-pass (_split_matmul_waits) moves extra waits onto
standalone single-wait EventSemaphore instructions inserted before the
offending instruction on the same engine queue.
"""

import numpy as np
import ml_dtypes
import os

def _B(name, default):
    return int(os.environ.get("KB_" + name, default))

N, B, D = 2048, 2, 512
H, R = 8, 3
C = D // H          # 64
NS = N // 4         # 512 query rows per core
NCORES = 8
MT = N // 128       # 16 key tiles
NP = 4              # mask planes staged per mt: [M0, M1, M2, M0]
ND = 6              # fp8 diag slots per head (3 DoubleRow pairs)

_cache = {}


def _build_program(reps=1, dr3=(True,) * H):
    import concourse.bass as bass
    import concourse.mybir as mybir
    import concourse.tile as tile
    from contextlib import ExitStack

    f32 = mybir.dt.float32
    f32r = mybir.dt.float32r
    bf16 = mybir.dt.bfloat16
    f8 = mybir.dt.float8e4
    AFT = mybir.ActivationFunctionType
    ALU = mybir.AluOpType

    nc = bass.Bass()

    xtq8 = nc.declare_dram_parameter("xtq8", [D, NS], f8, isOutput=False)
    xtqr = nc.declare_dram_parameter("xtqr", [D, NS], f8, isOutput=False)
    xtk8 = nc.declare_dram_parameter("xtk8", [D, N], f8, isOutput=False)
    xtkr = nc.declare_dram_parameter("xtkr", [D, N], f8, isOutput=False)
    xtv8 = nc.declare_dram_parameter("xtv8", [D, N], f8, isOutput=False)
    xtvr = nc.declare_dram_parameter("xtvr", [D, N], f8, isOutput=False)
    masksT = nc.declare_dram_parameter("masksT", [N, NP, NS], f8, isOutput=False)
    wq8 = nc.declare_dram_parameter("wq8", [D, D], f8, isOutput=False)
    wqr = nc.declare_dram_parameter("wqr", [D, D], f8, isOutput=False)
    wk8 = nc.declare_dram_parameter("wk8", [D, D], f8, isOutput=False)
    wkr = nc.declare_dram_parameter("wkr", [D, D], f8, isOutput=False)
    wv8 = nc.declare_dram_parameter("wv8", [D, D], f8, isOutput=False)
    wvr = nc.declare_dram_parameter("wvr", [D, D], f8, isOutput=False)
    wo = nc.declare_dram_parameter("wo", [D, D], bf16, isOutput=False)
    id8 = nc.declare_dram_parameter("id8", [128, H * ND * 128], f8, isOutput=False)
    bq2 = nc.declare_dram_parameter("bq2", [128, 4], f32, isOutput=False)
    bo2 = nc.declare_dram_parameter("bo2", [128, 4], f32, isOutput=False)
    pad = nc.declare_dram_parameter("pad", [128, MT], f32, isOutput=False)
    pad8 = nc.declare_dram_parameter("pad8", [128, MT, H], f32, isOutput=False)
    onesd = nc.declare_dram_parameter("onesd", [1, 64], f32r, isOutput=False)
    outT = nc.declare_dram_parameter("outT", [D, NS], bf16, isOutput=True)

    with tile.TileContext(nc) as tc, ExitStack() as ctx:
        mm = nc.tensor.matmul

        for _rep in range(reps):
            _run_once(nc, tc, ctx, mm, tile, mybir, f32, f32r, bf16, f8,
                      AFT, ALU, xtq8, xtqr, xtk8, xtkr, xtv8, xtvr, masksT,
                      wq8, wqr, wk8, wkr, wv8, wvr, wo,
                      id8, bq2, bo2, pad, pad8, onesd, outT, dr3)

    _split_matmul_waits(nc, mybir)
    return nc


def _run_once(nc, tc, ctx, mm, tile, mybir, f32, f32r, bf16, f8, AFT, ALU,
              xtq8, xtqr, xtk8, xtkr, xtv8, xtvr, masksT,
              wq8, wqr, wk8, wkr, wv8, wvr,
              wo, id8, bq2, bo2, pad, pad8, onesd, outT, dr3):
    from contextlib import ExitStack
    DR = mybir.MatmulPerfMode.DoubleRow
    with ExitStack() as ctx:
        const_pool = ctx.enter_context(tc.tile_pool(name="const", bufs=1))
        persist = ctx.enter_context(tc.tile_pool(name="persist", bufs=1))

        id_sb = const_pool.tile([128, H * ND, 128], f8)
        bq_sb = const_pool.tile([128, 4], f32)
        bo_sb = const_pool.tile([128, 4], f32)
        pad_sb = const_pool.tile([128, MT], f32)
        pad8_sb = const_pool.tile([128, MT, H], f32)

        ones_sb = const_pool.tile([1, 64], f32r)
        nc.sync.dma_start(ones_sb[:], onesd[:])
        wo_sb = persist.tile([128, 4, D], bf16)

        # mask planes, fp8, staged [M0, M1, M2, M0] per mt
        mall = persist.tile([128, MT, NP, NS], f8, name="mall")
        kT_sb = persist.tile([128, 4, N], bf16)
        qT_sb = persist.tile([128, 4, NS], bf16)
        v_sb = persist.tile([128, MT, H, C + 1], bf16)
        OT_sb = persist.tile([128, 4, NS], bf16)
        outT_sb = persist.tile([128, 4, NS], bf16)

        # DRAM views with the key dim on partitions
        masksTr = masksT.rearrange("(t p) d n -> p t d n", p=128)
        # column-block views of the 512-row x/w tensors: [p, c, cols]
        xtk8r = xtk8.rearrange("(c p) m -> p c m", p=128)
        xtkrr = xtkr.rearrange("(c p) m -> p c m", p=128)
        xtv8r = xtv8.rearrange("(c p) m -> p c m", p=128)
        xtvrr = xtvr.rearrange("(c p) m -> p c m", p=128)
        wor = wo.rearrange("(c p) d -> p c d", p=128)

        # V-projection operands persist into phase B (V is interleaved with
        # pass 0 there)
        vw_pool = ctx.enter_context(tc.tile_pool(name="vw", bufs=1))
        wv8_sb = vw_pool.tile([128, 4, D], f8, tag="wv8")
        wvr_sb = vw_pool.tile([128, 4, D], f8, tag="wvr")
        xv8_sb = vw_pool.tile([128, 4, N], f8, tag="xv8")
        xvr_sb = vw_pool.tile([128, 4, N], f8, tag="xvr")

        # ---- Phase A: q/k projections ----
        with tc.tile_pool(name="xw", bufs=1) as xw_pool, \
             tc.tile_pool(name="psA", bufs=_B("PSA", 8), space="PSUM") as psA:
            wq8_sb = xw_pool.tile([128, 4, D], f8, tag="w")
            wqr_sb = xw_pool.tile([128, 4, D], f8, tag="wr")
            wk8_sb = xw_pool.tile([128, 4, D], f8, tag="wk8")
            wkr_sb = xw_pool.tile([128, 4, D], f8, tag="wkr")
            xq8_sb = xw_pool.tile([128, 4, NS], f8, tag="xq8")
            xqr_sb = xw_pool.tile([128, 4, NS], f8, tag="xqr")
            xk8_sb = xw_pool.tile([128, 4, N], f8, tag="xk8")
            xkr_sb = xw_pool.tile([128, 4, N], f8, tag="xkr")

            # loads ordered by first use; V operands and mask quads 1-3
            # stream in during phase B's first pass
            # first operands on both DMA queues so their descriptor
            # processing overlaps
            nc.sync.dma_start(xq8_sb[:], xtq8.rearrange("(c p) n -> p c n", p=128))
            nc.gpsimd.dma_start(wq8_sb[:], wq8.rearrange("(c p) d -> p c d", p=128))
            nc.gpsimd.dma_start(bq_sb[:], bq2[:])
            nc.sync.dma_start(xqr_sb[:], xtqr.rearrange("(c p) n -> p c n", p=128))
            nc.sync.dma_start(wqr_sb[:], wqr.rearrange("(c p) d -> p c d", p=128))
            nc.sync.dma_start(wk8_sb[:], wk8.rearrange("(c p) d -> p c d", p=128))
            for mb in range(4):
                sl = slice(mb * 512, (mb + 1) * 512)
                nc.sync.dma_start(xk8_sb[:, :, sl], xtk8r[:, :, sl])
            nc.sync.dma_start(wkr_sb[:], wkr.rearrange("(c p) d -> p c d", p=128))
            for mb in range(4):
                sl = slice(mb * 512, (mb + 1) * 512)
                nc.sync.dma_start(xkr_sb[:, :, sl], xtkrr[:, :, sl])
            nc.sync.dma_start(pad_sb[:], pad[:])
            nc.sync.dma_start(pad8_sb[:], pad8[:])
            nc.sync.dma_start(wv8_sb[:], wv8.rearrange("(c p) d -> p c d", p=128))
            nc.sync.dma_start(wvr_sb[:], wvr.rearrange("(c p) d -> p c d", p=128))
            nc.sync.dma_start(xv8_sb[:, :, 0:512], xtv8r[:, :, 0:512])
            nc.sync.dma_start(xvr_sb[:, :, 0:512], xtvrr[:, :, 0:512])
            nc.sync.dma_start(mall[:, 0:2, :, :], masksTr[:, 0:2, :, :])
            nc.sync.dma_start(id_sb[:], id8.rearrange("p (i m) -> p i m", m=128))
            nc.sync.dma_start(mall[:, 2:4, :, :], masksTr[:, 2:4, :, :])
            for mb in range(1, 4):
                sl = slice(mb * 512, (mb + 1) * 512)
                nc.sync.dma_start(xv8_sb[:, :, sl], xtv8r[:, :, sl])
                nc.sync.dma_start(xvr_sb[:, :, sl], xtvrr[:, :, sl])
                nc.sync.dma_start(mall[:, 4 * mb:4 * mb + 4, :, :],
                                  masksTr[:, 4 * mb:4 * mb + 4, :, :])
            nc.sync.dma_start(bo_sb[:], bo2[:])
            nc.sync.dma_start(wo_sb[:], wor[:])

            vones = [nc.vector.tensor_copy(
                v_sb[:, :, :, C : C + 1],
                pad8_sb[:, :, :].rearrange("p m (h o) -> p m h o", o=1))]

            projc = []
            # qT[dh, n] = (16*Wq).T @ xT_q / 128  (+ bq/8 per-partition):
            # split-fp8 DoubleRows like kT; the epilogue's 1/128 scale
            # combines the 1/16 staging with the 1/sqrt(C) score scale
            for j in range(4):
                ps = psA.tile([128, NS], f32, tag="psA")
                first = True
                for wsb, xsb in ((wq8_sb, xq8_sb), (wq8_sb, xqr_sb),
                                 (wqr_sb, xq8_sb)):
                    for g in range(2):
                        gs = slice(2 * g, 2 * g + 2)
                        mm(ps[:], wsb[:, gs, j * 128:(j + 1) * 128],
                           xsb[:, gs, :], start=first,
                           stop=(wsb is wqr_sb and g == 1), perf_mode=DR)
                        first = False
                projc.append(nc.scalar.activation(
                    qT_sb[:, j, :], ps[:], AFT.Identity,
                    bias=bq_sb[:, j:j + 1], scale=1.0 / 128.0))

            # kT[dh, m] = (16*Wk).T @ xT_k / 16 via split-fp8 DoubleRows:
            # W = w8 + wr, x = x8 + xr; accumulate w8*x8 + w8*xr + wr*x8
            # (the wr*xr term is ~0.1% and dropped); epilogue scales 1/16.
            # (k bias is softmax-invariant: dropped)
            for mb in range(4):
                for j in range(4):
                    ps = psA.tile([128, NS], f32, tag="psA")
                    first = True
                    for wsb, xsb in ((wk8_sb, xk8_sb), (wk8_sb, xkr_sb),
                                     (wkr_sb, xk8_sb)):
                        for g in range(2):
                            gs = slice(2 * g, 2 * g + 2)
                            mm(ps[:], wsb[:, gs, j * 128:(j + 1) * 128],
                               xsb[:, gs, mb * 512:(mb + 1) * 512],
                               start=first,
                               stop=(wsb is wkr_sb and g == 1), perf_mode=DR)
                            first = False
                    if j < 2:
                        projc.append(nc.scalar.activation(
                            kT_sb[:, j, mb * 512:(mb + 1) * 512], ps[:],
                            AFT.Identity, scale=0.0625))
                    else:
                        projc.append(nc.vector.tensor_scalar(
                            kT_sb[:, j, mb * 512:(mb + 1) * 512], ps[:],
                            0.0625, None, ALU.mult))

        # PSUM pools for phase B (after phase A's psA released its banks)
        psO = ctx.enter_context(tc.tile_pool(name="psO", bufs=_B("PSO", 2), space="PSUM"))
        psS = ctx.enter_context(tc.tile_pool(name="psS", bufs=_B("PSS", 3), space="PSUM"))

        pT_pool = ctx.enter_context(tc.tile_pool(name="pT", bufs=_B("PT", 3)))
        small_pool = ctx.enter_context(tc.tile_pool(name="small", bufs=8))

        # ---- Phase B: attention, four passes of 2 heads ----
        def make_norm(p, heads, o_ps, last=False):
            def emit():
                rsbs = []
                for ih in range(2):
                    rsb = small_pool.tile([1, NS], f32r, tag="rsb",
                                          name=f"rsb{p}_{ih}")
                    # f32r is bit-identical to f32; it only switches the PE
                    # broadcast matmul to 1-cycle/row streaming
                    with nc.allow_low_precision(reason="f32r == f32 bits"):
                        nc.vector.reciprocal(rsb[:], o_ps[ih][64:65, :])
                    rsbs.append(rsb)
                b_ps = psS.tile([128, 2, NS], f32, tag="psS", name=f"bps{p}")
                for ih in range(2):
                    mm(b_ps[0:64, ih, :], ones_sb[0:1, :], rsbs[ih][0:1, :],
                       start=True, stop=True)
                b_sbs = []
                for ih in range(2):
                    b_sb = small_pool.tile([64, NS], f32, tag="bsb",
                                           name=f"bsb{p}_{ih}")
                    # in the drain tail the Act engine is idle: put the
                    # PSUM->SBUF hop there so the OT multiplies overlap it
                    if last:
                        nc.scalar.copy(b_sb[:], b_ps[0:64, ih, :])
                    else:
                        nc.vector.tensor_copy(b_sb[:], b_ps[0:64, ih, :])
                    b_sbs.append(b_sb)
                for ih, h in enumerate(heads):
                    hj, ho = h // 2, (h % 2) * 64
                    nc.vector.tensor_tensor(
                        OT_sb[ho:ho + 64, hj, :], o_ps[ih][0:64, :],
                        b_sbs[ih][:], ALU.mult)
            return emit

        # v[m, c] = xT_v.T @ (16*Wv) / 16, split-fp8 like kT; padded key
        # rows zeroed (pad01/16 folded into the host-side pad tensor).
        # Emitted inside pass 0, one pair ahead of its attn@V consumer, so
        # phase B starts right after the k-projection.
        def emit_v(mt):
            ps = psS.tile([128, 2, NS], f32, tag="psS", name=f"psV{mt}")
            first = True
            for xsb, wsb in ((xv8_sb, wv8_sb), (xv8_sb, wvr_sb),
                             (xvr_sb, wv8_sb)):
                for g in range(2):
                    gs = slice(2 * g, 2 * g + 2)
                    mm(ps[:, 0, :], xsb[:, gs, mt * 128:(mt + 1) * 128],
                       wsb[:, gs, :], start=first,
                       stop=(xsb is xvr_sb and g == 1), perf_mode=DR)
                    first = False
            nc.vector.tensor_scalar(
                v_sb[:, mt, :, 0:C],
                ps[:, 0, :].rearrange("p (h c) -> p h c", h=H),
                pad_sb[:, mt:mt + 1], None, ALU.mult)

        pending_norm = None
        v_emitted = 0
        for p in range(4):
            heads = (2 * p, 2 * p + 1)
            o_ps = [psO.tile([128, NS], f32, tag="psO", name=f"o_ps{p}_{i}")
                    for i in range(2)]
            for pr in range(MT // 2):
                if p == 0:
                    while v_emitted < min(MT, 2 * pr + _B("VLA", 4)):
                        emit_v(v_emitted)
                        v_emitted += 1
                for ih, h in enumerate(heads):
                    hj, ho = h // 2, (h % 2) * 64
                    s2 = psS.tile([128, 2, NS], f32, tag="psS")
                    # heads with small fp8 coefficient residuals drop the
                    # middle DoubleRow (pairs (M0,M1) + (M2,M0dup) suffice)
                    d_list = (0, 1, 2) if dr3[h] else (0, 2)
                    for i in range(2):
                        mt = 2 * pr + i
                        mm(s2[:, i, :],
                           kT_sb[ho:ho + 64, hj, mt * 128:(mt + 1) * 128],
                           qT_sb[ho:ho + 64, hj, :], start=True, stop=False)
                        for d in d_list:
                            mm(s2[:, i, :],
                               id_sb[:, h * ND + 2 * d : h * ND + 2 * d + 2, :],
                               mall[:, mt, d:d + 2, :],
                               start=False, stop=(d == d_list[-1]),
                               perf_mode=DR)
                    pT = pT_pool.tile([128, 2, NS], bf16, tag="pT")
                    nc.scalar.activation(pT[:], s2[:], AFT.Exp)
                    for i in range(2):
                        mt = 2 * pr + i
                        mm(o_ps[ih][0:65, :], v_sb[:, mt, h, :], pT[:, i, :],
                           start=(mt == 0), stop=(mt == MT - 1))
                if pr == 0 and pending_norm is not None:
                    # previous pass's normalization: the DVE reciprocal
                    # latency hides under this pass's first-pair PE work
                    pending_norm()
                    pending_norm = None
            pending_norm = make_norm(p, heads, o_ps, last=(p == 3))

        # ---- Phase C: output projection ----
        # last normalization's PE broadcasts go first (they park while the
        # g=0..2 pre-accumulation runs); only the g=3 matmuls and epilogues
        # sit in the drain tail
        pending_norm()
        psC = [psS.tile([128, 2, NS], f32, tag="psS", name=f"psC{a}")
               for a in range(2)]
        for jt in range(4):
            ps = psC[jt // 2][:, jt % 2, :]
            for g in range(3):
                mm(ps, wo_sb[:, g, jt * 128:(jt + 1) * 128],
                   OT_sb[:, g, :], start=(g == 0), stop=False)
        for jt in range(4):
            ps = psC[jt // 2][:, jt % 2, :]
            mm(ps, wo_sb[:, 3, jt * 128:(jt + 1) * 128],
               OT_sb[:, 3, :], start=False, stop=True)
            # split the bias epilogues across Act and DVE so the four
            # drains pipeline two-wide
            if jt % 2 == 0:
                nc.scalar.activation(outT_sb[:, jt, :], ps, AFT.Identity,
                                     bias=bo_sb[:, jt:jt + 1])
            else:
                nc.vector.tensor_scalar(outT_sb[:, jt, :], ps,
                                        bo_sb[:, jt:jt + 1], None, ALU.add)
            # alternate DMA queues so the four drains overlap
            eng = nc.sync if jt % 2 == 0 else nc.gpsimd
            eng.dma_start(outT[jt * 128:(jt + 1) * 128, :],
                          outT_sb[:, jt, :])


# every TPB instruction encoding in this walrus build tolerates only a
# single semaphore wait -- split extras regardless of opcode
_NO_SPLIT_TYPES = {"InstEventSemaphore"}


def _split_matmul_waits(nc, mybir):
    """Several engine instruction encodings tolerate only one semaphore
    wait; move extra waits onto standalone single-wait EventSemaphore
    instructions inserted right before them on the same engine queue."""
    import bass_rust

    n = 0
    for bb in nc.m.functions[0].blocks:
        insts = list(bb.instructions)
        out = []
        changed = False
        for i in insts:
            si = i.sync_info
            if (type(i).__name__ not in _NO_SPLIT_TYPES and si is not None
                    and len(si.on_wait) > 1):
                w = list(si.on_wait)
                for wx in w[:-1]:
                    ev = mybir.InstEventSemaphore(name=f"mmw_{n}_{i.name}",
                                                  ins=[], outs=[])
                    ev.engine = i.engine
                    ev.sync_info = bass_rust.SyncInfo(on_wait=[wx],
                                                      on_update=[])
                    out.append(ev)
                    n += 1
                si.on_wait = [w[-1]]
                changed = True
            out.append(i)
        if changed:
            bb.instructions = out


def _pick_dr3(absres, c):
    """Give the full 3-DoubleRow (exact-residual) mask path to the K3
    heads where dropping the r1/r2 residuals hurts most.  A coefficient
    error delta_r shifts the weight of class-r keys, whose share of the
    softmax mass scales as exp(-c_r), so impact ~ |delta_r|*exp(-c_r).
    The rest run the 2-DoubleRow variant (r0 still corrected via the
    duplicate M0 plane)."""
    k3 = _B("K3", 0)
    score = np.sum(absres[:, 1:] * np.exp(-c[:, 1:]), axis=1)
    order = np.argsort(score)[::-1]
    dr3 = [False] * H
    for h in order[:k3]:
        dr3[int(h)] = True
    return tuple(dr3)


def _host_prep(inputs):
    x_q = np.asarray(inputs["x_q"], np.float32)
    x_k = np.asarray(inputs["x_k"], np.float32)
    x_v = np.asarray(inputs["x_v"], np.float32)
    attn_mask = np.asarray(inputs["attn_mask"]).astype(bool)
    kpm = np.asarray(inputs["key_padding_mask"]).astype(bool)
    Wq = np.asarray(inputs["Wq"], np.float32)
    Wk = np.asarray(inputs["Wk"], np.float32)
    Wv = np.asarray(inputs["Wv"], np.float32)
    Wo = np.asarray(inputs["Wo"], np.float32)
    bq = np.asarray(inputs["bq"], np.float32)
    bv = np.asarray(inputs["bv"], np.float32)
    bo = np.asarray(inputs["bo"], np.float32)
    mw = np.asarray(inputs["mask_weight"], np.float64)

    # c[h,r] = softmax(mask_weight[h,:R]) * mask_weight[h,R]
    e = np.exp(mw[:, :R] - mw[:, :R].max(axis=1, keepdims=True))
    w = e / e.sum(axis=1, keepdims=True)
    c = (w * mw[:, R:R + 1]).astype(np.float64)          # [H, R]

    # split each coefficient into fp8 main + fp8 residual; DoubleRow d
    # covers plane pair (d, d+1) of the staged planes [M0, M1, M2, M0dup].
    # 3-DR heads apply all six products [c0, c1, r1, c2, r2, r0]; heads
    # whose r1/r2 residuals are negligible use only DRs 0 and 2 with
    # slots [c0, c1, -, -, c2, r0].
    fp8 = ml_dtypes.float8_e4m3
    cm = c.astype(fp8)                                    # main
    cr = (c - cm.astype(np.float64)).astype(fp8)          # residual
    cmf = cm.astype(np.float32)
    crf = cr.astype(np.float32)
    dr3 = _pick_dr3(np.abs(c - cm.astype(np.float64)), c)
    slots = np.zeros((H, ND), np.float32)
    for h in range(H):
        if dr3[h]:
            slots[h] = [cmf[h, 0], cmf[h, 1], crf[h, 1],
                        cmf[h, 2], crf[h, 2], crf[h, 0]]
        else:
            slots[h] = [cmf[h, 0], cmf[h, 1], 0.0,
                        0.0, cmf[h, 2], crf[h, 0]]

    id8 = np.zeros((H * ND, 128, 128), np.float32)
    eye = np.eye(128, dtype=np.float32)
    for h in range(H):
        for d in range(ND):
            id8[h * ND + d] = eye * (-slots[h, d])
    # partition-major so the DMA is one contiguous descriptor per row
    id8 = np.ascontiguousarray(
        id8.transpose(1, 0, 2)).reshape(128, H * ND * 128).astype(fp8)

    scale = np.float32(1.0 / np.sqrt(C))
    bq_s = (bq * scale).astype(np.float32)
    bo_p = (bv @ Wo + bo).astype(np.float32)

    bq2 = np.ascontiguousarray(bq_s.reshape(4, 128).T)
    bo2 = np.ascontiguousarray(bo_p.reshape(4, 128).T)

    def split8(a):
        # split a (in the 16x domain for weights) into fp8 main + residual
        hi = a.astype(fp8)
        lo = (a - hi.astype(np.float32)).astype(fp8)
        return hi, lo

    bf = ml_dtypes.bfloat16
    wq8, wqr = split8(np.ascontiguousarray(Wq) * np.float32(16.0))
    wk8, wkr = split8(np.ascontiguousarray(Wk) * np.float32(16.0))
    wv8, wvr = split8(np.ascontiguousarray(Wv) * np.float32(16.0))
    common = dict(wq8=wq8, wqr=wqr, wk8=wk8, wkr=wkr, wv8=wv8, wvr=wvr,
                  wo=Wo.astype(bf), id8=id8, bq2=bq2, bo2=bo2,
                  onesd=np.ones((1, 64), np.float32))

    in_maps = []
    for core in range(NCORES):
        b, ns = core // 4, core % 4
        n0 = ns * NS
        pad01 = (~kpm[b]).astype(np.float32) * np.float32(1.0 / 16.0)  # [N]
        pad2 = np.ascontiguousarray(pad01.reshape(MT, 128).T)
        pad8 = np.ascontiguousarray(
            np.repeat((pad2 != 0).astype(np.float32)[:, :, None], H, axis=2))
        m = dict(common)
        m["xtq8"], m["xtqr"] = split8(
            np.ascontiguousarray(x_q[n0:n0 + NS, b, :].T))
        xkT = np.ascontiguousarray(x_k[:, b, :].T)
        xvT = np.ascontiguousarray(x_v[:, b, :].T)
        m["xtk8"], m["xtkr"] = split8(xkT)
        m["xtv8"], m["xtvr"] = split8(xvT)
        m3 = attn_mask[b, :, n0:n0 + NS, :].transpose(2, 0, 1)  # [m, R, n]
        m4 = np.concatenate([m3, m3[:, 0:1]], axis=1)           # [m, 4, n]
        m["masksT"] = np.ascontiguousarray(m4).astype(np.float32).astype(fp8)
        m["pad"] = pad2
        m["pad8"] = pad8
        in_maps.append(m)
    return in_maps, dr3


def kernel(**inputs) -> np.ndarray:
    from concourse.bass_utils import run_bass_kernel_spmd

    in_maps, dr3 = _host_prep(inputs)
    if _cache.get("dr3") != dr3:
        _cache["nc"] = _build_program(dr3=dr3)
        _cache["dr3"] = dr3
    nc = _cache["nc"]

    res = run_bass_kernel_spmd(nc, in_maps, list(range(NCORES)))

    out = np.empty((N, B, D), np.float32)
    for core in range(NCORES):
        b, ns = core // 4, core % 4
        n0 = ns * NS
        out[n0:n0 + NS, b, :] = res.results[core]["outT"].astype(np.float32).T
    return out


# revision 84
# speedup vs baseline: 1.0026x; 1.0026x over previous
"""Bass/Trainium2 kernel for DynamicMultiheadAttention (sparse_attention).

Sharding: 8 cores = (batch b in {0,1}) x (query-slice of 512 rows).
Each core computes all 8 heads for its (b, n-slice) in transposed
orientation: scores sT[m, n] with keys m on partitions, so that
  - the relative-mask bias  -sum_r c[h,r]*M_r[m,n]  is accumulated into
    score PSUM by fp8 DoubleRow matmuls (0.5 cycles/row, two mask
    planes per instruction) with scaled-identity stationary operands.
    Masks are 0/1 (exact in fp8); each coefficient is split
    c = fp8(c) + fp8(c - fp8(c)).  Mask planes are staged as
    [M0,M1,M2,M0] so DoubleRow pairs (0,1),(1,2),(2,3) can cover each
    plane twice (main + residual) with stride-adjacent slices.  By
    default every head runs just two DoubleRows -- pairs (0,1) and
    (2,3), i.e. all mains plus the r0 residual -- which costs ~1.1e-2
    end-to-end vs the 2e-2 budget; KB_K3=n instead gives the n heads
    with the largest |residual|*exp(-c) impact the full three-DoubleRow
    (exact to ~4e-3) path.
  - softmax row-sums come free from a ones-column appended to V,
  - attn @ V needs no transposes (pT tiles are directly the stationary
    operand layout).
Key padding is applied by zeroing padded key rows of V and of the
ones-column (exactly equivalent to -inf logits). The row-constant term
scale_h * sum_r w[h,r] = scale_h cancels in softmax and is dropped; the
k-projection bias is softmax-invariant and dropped; the v bias folds
into the output bias (softmax rows sum to 1): bo' = bv @ Wo + bo.

The q/k/v projections run as split-fp8 DoubleRows: the host stages
16*W = w8 + wr and x = x8 + xr (two fp8e4 tensors each, same bytes as
bf16), the kernel accumulates w8x8 + w8xr + wrx8 over 256-deep k-tile
pairs (the wrxr term is ~0.1% and dropped) and the epilogue scales by
1/16 (1/128 for q, folding in the 1/sqrt(C) score scale) -- ~2.4x
fewer PE cycles than bf16 at bf16-level accuracy.  The 16x staging
keeps the weight residuals out of fp8's subnormal range.

Attention runs in 4 passes of 2 heads so PSUM fits: per (head,
mt-pair) a [128,2,512] score tile (2 banks) accumulates QK (bf16) plus
the mask DoubleRows, one Exp activation covers both tiles (halves the
Activation-engine instruction count), and two bf16 attn@V matmuls
drain it into the per-head output accumulator.  The v-projection is
emitted inside pass 0, one pair ahead of its attn@V consumer, so phase
B starts right after the k-projection.  Each pass's normalization is
emitted after the next pass's first pair so the DVE reciprocal latency
hides under PE work; the output projection pre-accumulates heads 0-5
before the last normalization and only the g=3 matmuls, split Act/DVE
epilogues, and dual-queue output DMAs sit in the drain tail.

DMA schedule is ordered by first use (xtq/wq -> wk/xtk -> masks/id8
interleaved with wv/xtv -> wo), with the 512-row tensors loaded in
full-height column blocks so each projection sub-block starts as soon
as its operands land.

Measured on the staged harness: rel err 1.11e-2, HW exec 131356 ns
(timeline-sim estimate; baseline was 200796 ns).

Every TPB instruction encoding in this walrus build tolerates only ONE
semaphore wait; a post-pass (_split_matmul_waits) moves extra waits onto
standalone single-wait EventSemaphore instructions inserted before the
offending instruction on the same engine queue.
"""

import numpy as np
import ml_dtypes
import os

def _B(name, default):
    return int(os.environ.get("KB_" + name, default))

N, B, D = 2048, 2, 512
H, R = 8, 3
C = D // H          # 64
NS = N // 4         # 512 query rows per core
NCORES = 8
MT = N // 128       # 16 key tiles
NP = 4              # mask planes staged per mt: [M0, M1, M2, M0]
ND = 6              # fp8 diag slots per head (3 DoubleRow pairs)

_cache = {}


def _build_program(reps=1, dr3=(True,) * H):
    import concourse.bass as bass
    import concourse.mybir as mybir
    import concourse.tile as tile
    from contextlib import ExitStack

    f32 = mybir.dt.float32
    f32r = mybir.dt.float32r
    bf16 = mybir.dt.bfloat16
    f8 = mybir.dt.float8e4
    AFT = mybir.ActivationFunctionType
    ALU = mybir.AluOpType

    nc = bass.Bass()

    xtq8 = nc.declare_dram_parameter("xtq8", [D, NS], f8, isOutput=False)
    xtqr = nc.declare_dram_parameter("xtqr", [D, NS], f8, isOutput=False)
    xtk8 = nc.declare_dram_parameter("xtk8", [D, N], f8, isOutput=False)
    xtkr = nc.declare_dram_parameter("xtkr", [D, N], f8, isOutput=False)
    xtv8 = nc.declare_dram_parameter("xtv8", [D, N], f8, isOutput=False)
    xtvr = nc.declare_dram_parameter("xtvr", [D, N], f8, isOutput=False)
    masksT = nc.declare_dram_parameter("masksT", [N, NP, NS], f8, isOutput=False)
    wq8 = nc.declare_dram_parameter("wq8", [D, D], f8, isOutput=False)
    wqr = nc.declare_dram_parameter("wqr", [D, D], f8, isOutput=False)
    wk8 = nc.declare_dram_parameter("wk8", [D, D], f8, isOutput=False)
    wkr = nc.declare_dram_parameter("wkr", [D, D], f8, isOutput=False)
    wv8 = nc.declare_dram_parameter("wv8", [D, D], f8, isOutput=False)
    wvr = nc.declare_dram_parameter("wvr", [D, D], f8, isOutput=False)
    wo = nc.declare_dram_parameter("wo", [D, D], bf16, isOutput=False)
    id8 = nc.declare_dram_parameter("id8", [128, H * ND * 128], f8, isOutput=False)
    bq2 = nc.declare_dram_parameter("bq2", [128, 4], f32, isOutput=False)
    bo2 = nc.declare_dram_parameter("bo2", [128, 4], f32, isOutput=False)
    pad = nc.declare_dram_parameter("pad", [128, MT], f32, isOutput=False)
    pad8 = nc.declare_dram_parameter("pad8", [128, MT, H], f32, isOutput=False)
    onesd = nc.declare_dram_parameter("onesd", [1, 64], f32r, isOutput=False)
    outT = nc.declare_dram_parameter("outT", [D, NS], bf16, isOutput=True)

    with tile.TileContext(nc) as tc, ExitStack() as ctx:
        mm = nc.tensor.matmul

        for _rep in range(reps):
            _run_once(nc, tc, ctx, mm, tile, mybir, f32, f32r, bf16, f8,
                      AFT, ALU, xtq8, xtqr, xtk8, xtkr, xtv8, xtvr, masksT,
                      wq8, wqr, wk8, wkr, wv8, wvr, wo,
                      id8, bq2, bo2, pad, pad8, onesd, outT, dr3)

    _split_matmul_waits(nc, mybir)
    return nc


def _run_once(nc, tc, ctx, mm, tile, mybir, f32, f32r, bf16, f8, AFT, ALU,
              xtq8, xtqr, xtk8, xtkr, xtv8, xtvr, masksT,
              wq8, wqr, wk8, wkr, wv8, wvr,
              wo, id8, bq2, bo2, pad, pad8, onesd, outT, dr3):
    from contextlib import ExitStack
    DR = mybir.MatmulPerfMode.DoubleRow
    with ExitStack() as ctx:
        const_pool = ctx.enter_context(tc.tile_pool(name="const", bufs=1))
        persist = ctx.enter_context(tc.tile_pool(name="persist", bufs=1))

        id_sb = const_pool.tile([128, H * ND, 128], f8)
        bq_sb = const_pool.tile([128, 4], f32)
        bo_sb = const_pool.tile([128, 4], f32)
        pad_sb = const_pool.tile([128, MT], f32)
        pad8_sb = const_pool.tile([128, MT, H], f32)

        ones_sb = const_pool.tile([1, 64], f32r)
        nc.sync.dma_start(ones_sb[:], onesd[:])
        wo_sb = persist.tile([128, 4, D], bf16)

        # mask planes, fp8, staged [M0, M1, M2, M0] per mt
        mall = persist.tile([128, MT, NP, NS], f8, name="mall")
        kT_sb = persist.tile([128, 4, N], bf16)
        qT_sb = persist.tile([128, 4, NS], bf16)
        v_sb = persist.tile([128, MT, H, C + 1], bf16)
        OT_sb = persist.tile([128, 4, NS], bf16)
        outT_sb = persist.tile([128, 4, NS], bf16)

        # DRAM views with the key dim on partitions
        masksTr = masksT.rearrange("(t p) d n -> p t d n", p=128)
        # column-block views of the 512-row x/w tensors: [p, c, cols]
        xtk8r = xtk8.rearrange("(c p) m -> p c m", p=128)
        xtkrr = xtkr.rearrange("(c p) m -> p c m", p=128)
        xtv8r = xtv8.rearrange("(c p) m -> p c m", p=128)
        xtvrr = xtvr.rearrange("(c p) m -> p c m", p=128)
        wor = wo.rearrange("(c p) d -> p c d", p=128)

        # V-projection operands persist into phase B (V is interleaved with
        # pass 0 there)
        vw_pool = ctx.enter_context(tc.tile_pool(name="vw", bufs=1))
        wv8_sb = vw_pool.tile([128, 4, D], f8, tag="wv8")
        wvr_sb = vw_pool.tile([128, 4, D], f8, tag="wvr")
        xv8_sb = vw_pool.tile([128, 4, N], f8, tag="xv8")
        xvr_sb = vw_pool.tile([128, 4, N], f8, tag="xvr")

        # ---- Phase A: q/k projections ----
        with tc.tile_pool(name="xw", bufs=1) as xw_pool, \
             tc.tile_pool(name="psA", bufs=_B("PSA", 8), space="PSUM") as psA:
            wq8_sb = xw_pool.tile([128, 4, D], f8, tag="w")
            wqr_sb = xw_pool.tile([128, 4, D], f8, tag="wr")
            wk8_sb = xw_pool.tile([128, 4, D], f8, tag="wk8")
            wkr_sb = xw_pool.tile([128, 4, D], f8, tag="wkr")
            xq8_sb = xw_pool.tile([128, 4, NS], f8, tag="xq8")
            xqr_sb = xw_pool.tile([128, 4, NS], f8, tag="xqr")
            xk8_sb = xw_pool.tile([128, 4, N], f8, tag="xk8")
            xkr_sb = xw_pool.tile([128, 4, N], f8, tag="xkr")

            # loads ordered by first use; V operands and mask quads 1-3
            # stream in during phase B's first pass
            # first operands on both DMA queues so their descriptor
            # processing overlaps
            xtq8r = xtq8.rearrange("(c p) n -> p c n", p=128)
            xtqrr = xtqr.rearrange("(c p) n -> p c n", p=128)
            wq8r = wq8.rearrange("(c p) d -> p c d", p=128)
            wqrr = wqr.rearrange("(c p) d -> p c d", p=128)
            # halves: the first Q DoubleRow needs only chunks 0-1 of each
            nc.sync.dma_start(xq8_sb[:, 0:2, :], xtq8r[:, 0:2, :])
            nc.gpsimd.dma_start(wq8_sb[:, 0:2, :], wq8r[:, 0:2, :])
            nc.gpsimd.dma_start(bq_sb[:], bq2[:])
            nc.sync.dma_start(xq8_sb[:, 2:4, :], xtq8r[:, 2:4, :])
            nc.gpsimd.dma_start(wq8_sb[:, 2:4, :], wq8r[:, 2:4, :])
            nc.sync.dma_start(xqr_sb[:], xtqrr[:])
            nc.gpsimd.dma_start(wqr_sb[:], wqrr[:])
            nc.sync.dma_start(wk8_sb[:], wk8.rearrange("(c p) d -> p c d", p=128))
            for mb in range(4):
                sl = slice(mb * 512, (mb + 1) * 512)
                nc.sync.dma_start(xk8_sb[:, :, sl], xtk8r[:, :, sl])
            nc.sync.dma_start(wkr_sb[:], wkr.rearrange("(c p) d -> p c d", p=128))
            for mb in range(4):
                sl = slice(mb * 512, (mb + 1) * 512)
                nc.sync.dma_start(xkr_sb[:, :, sl], xtkrr[:, :, sl])
            nc.sync.dma_start(pad_sb[:], pad[:])
            nc.sync.dma_start(pad8_sb[:], pad8[:])
            nc.sync.dma_start(wv8_sb[:], wv8.rearrange("(c p) d -> p c d", p=128))
            nc.sync.dma_start(wvr_sb[:], wvr.rearrange("(c p) d -> p c d", p=128))
            nc.sync.dma_start(xv8_sb[:, :, 0:512], xtv8r[:, :, 0:512])
            nc.sync.dma_start(xvr_sb[:, :, 0:512], xtvrr[:, :, 0:512])
            nc.sync.dma_start(mall[:, 0:2, :, :], masksTr[:, 0:2, :, :])
            nc.sync.dma_start(id_sb[:], id8.rearrange("p (i m) -> p i m", m=128))
            nc.sync.dma_start(mall[:, 2:4, :, :], masksTr[:, 2:4, :, :])
            for mb in range(1, 4):
                sl = slice(mb * 512, (mb + 1) * 512)
                nc.sync.dma_start(xv8_sb[:, :, sl], xtv8r[:, :, sl])
                nc.sync.dma_start(xvr_sb[:, :, sl], xtvrr[:, :, sl])
                nc.sync.dma_start(mall[:, 4 * mb:4 * mb + 4, :, :],
                                  masksTr[:, 4 * mb:4 * mb + 4, :, :])
            nc.sync.dma_start(bo_sb[:], bo2[:])
            nc.sync.dma_start(wo_sb[:], wor[:])

            vones = [nc.vector.tensor_copy(
                v_sb[:, :, :, C : C + 1],
                pad8_sb[:, :, :].rearrange("p m (h o) -> p m h o", o=1))]

            projc = []
            # qT[dh, n] = (16*Wq).T @ xT_q / 128  (+ bq/8 per-partition):
            # split-fp8 DoubleRows like kT; the epilogue's 1/128 scale
            # combines the 1/16 staging with the 1/sqrt(C) score scale
            for j in range(4):
                ps = psA.tile([128, NS], f32, tag="psA")
                first = True
                for wsb, xsb in ((wq8_sb, xq8_sb), (wq8_sb, xqr_sb),
                                 (wqr_sb, xq8_sb)):
                    for g in range(2):
                        gs = slice(2 * g, 2 * g + 2)
                        mm(ps[:], wsb[:, gs, j * 128:(j + 1) * 128],
                           xsb[:, gs, :], start=first,
                           stop=(wsb is wqr_sb and g == 1), perf_mode=DR)
                        first = False
                projc.append(nc.scalar.activation(
                    qT_sb[:, j, :], ps[:], AFT.Identity,
                    bias=bq_sb[:, j:j + 1], scale=1.0 / 128.0))

            # kT[dh, m] = (16*Wk).T @ xT_k / 16 via split-fp8 DoubleRows:
            # W = w8 + wr, x = x8 + xr; accumulate w8*x8 + w8*xr + wr*x8
            # (the wr*xr term is ~0.1% and dropped); epilogue scales 1/16.
            # (k bias is softmax-invariant: dropped)
            for mb in range(4):
                for j in range(4):
                    ps = psA.tile([128, NS], f32, tag="psA")
                    first = True
                    for wsb, xsb in ((wk8_sb, xk8_sb), (wk8_sb, xkr_sb),
                                     (wkr_sb, xk8_sb)):
                        for g in range(2):
                            gs = slice(2 * g, 2 * g + 2)
                            mm(ps[:], wsb[:, gs, j * 128:(j + 1) * 128],
                               xsb[:, gs, mb * 512:(mb + 1) * 512],
                               start=first,
                               stop=(wsb is wkr_sb and g == 1), perf_mode=DR)
                            first = False
                    if j < 2:
                        projc.append(nc.scalar.activation(
                            kT_sb[:, j, mb * 512:(mb + 1) * 512], ps[:],
                            AFT.Identity, scale=0.0625))
                    else:
                        projc.append(nc.vector.tensor_scalar(
                            kT_sb[:, j, mb * 512:(mb + 1) * 512], ps[:],
                            0.0625, None, ALU.mult))

        # PSUM pools for phase B (after phase A's psA released its banks)
        psO = ctx.enter_context(tc.tile_pool(name="psO", bufs=_B("PSO", 2), space="PSUM"))
        psS = ctx.enter_context(tc.tile_pool(name="psS", bufs=_B("PSS", 3), space="PSUM"))

        pT_pool = ctx.enter_context(tc.tile_pool(name="pT", bufs=_B("PT", 3)))
        small_pool = ctx.enter_context(tc.tile_pool(name="small", bufs=8))

        # ---- Phase B: attention, four passes of 2 heads ----
        def make_norm(p, heads, o_ps, last=False):
            def emit():
                rsbs = []
                for ih in range(2):
                    rsb = small_pool.tile([1, NS], f32r, tag="rsb",
                                          name=f"rsb{p}_{ih}")
                    # f32r is bit-identical to f32; it only switches the PE
                    # broadcast matmul to 1-cycle/row streaming
                    with nc.allow_low_precision(reason="f32r == f32 bits"):
                        nc.vector.reciprocal(rsb[:], o_ps[ih][64:65, :])
                    rsbs.append(rsb)
                b_ps = psS.tile([128, 2, NS], f32, tag="psS", name=f"bps{p}")
                for ih in range(2):
                    mm(b_ps[0:64, ih, :], ones_sb[0:1, :], rsbs[ih][0:1, :],
                       start=True, stop=True)
                b_sbs = []
                for ih in range(2):
                    b_sb = small_pool.tile([64, NS], f32, tag="bsb",
                                           name=f"bsb{p}_{ih}")
                    # in the drain tail the Act engine is idle: put the
                    # PSUM->SBUF hop there so the OT multiplies overlap it
                    if last:
                        nc.scalar.copy(b_sb[:], b_ps[0:64, ih, :])
                    else:
                        nc.vector.tensor_copy(b_sb[:], b_ps[0:64, ih, :])
                    b_sbs.append(b_sb)
                for ih, h in enumerate(heads):
                    hj, ho = h // 2, (h % 2) * 64
                    nc.vector.tensor_tensor(
                        OT_sb[ho:ho + 64, hj, :], o_ps[ih][0:64, :],
                        b_sbs[ih][:], ALU.mult)
            return emit

        # v[m, c] = xT_v.T @ (16*Wv) / 16, split-fp8 like kT; padded key
        # rows zeroed (pad01/16 folded into the host-side pad tensor).
        # Emitted inside pass 0, one pair ahead of its attn@V consumer, so
        # phase B starts right after the k-projection.
        def emit_v(mt):
            ps = psS.tile([128, 2, NS], f32, tag="psS", name=f"psV{mt}")
            first = True
            for xsb, wsb in ((xv8_sb, wv8_sb), (xv8_sb, wvr_sb),
                             (xvr_sb, wv8_sb)):
                for g in range(2):
                    gs = slice(2 * g, 2 * g + 2)
                    mm(ps[:, 0, :], xsb[:, gs, mt * 128:(mt + 1) * 128],
                       wsb[:, gs, :], start=first,
                       stop=(xsb is xvr_sb and g == 1), perf_mode=DR)
                    first = False
            nc.vector.tensor_scalar(
                v_sb[:, mt, :, 0:C],
                ps[:, 0, :].rearrange("p (h c) -> p h c", h=H),
                pad_sb[:, mt:mt + 1], None, ALU.mult)

        pending_norm = None
        v_emitted = 0
        for p in range(4):
            heads = (2 * p, 2 * p + 1)
            o_ps = [psO.tile([128, NS], f32, tag="psO", name=f"o_ps{p}_{i}")
                    for i in range(2)]
            for pr in range(MT // 2):
                if p == 0:
                    while v_emitted < min(MT, 2 * pr + _B("VLA", 4)):
                        emit_v(v_emitted)
                        v_emitted += 1
                for ih, h in enumerate(heads):
                    hj, ho = h // 2, (h % 2) * 64
                    s2 = psS.tile([128, 2, NS], f32, tag="psS")
                    # heads with small fp8 coefficient residuals drop the
                    # middle DoubleRow (pairs (M0,M1) + (M2,M0dup) suffice)
                    d_list = (0, 1, 2) if dr3[h] else (0, 2)
                    for i in range(2):
                        mt = 2 * pr + i
                        mm(s2[:, i, :],
                           kT_sb[ho:ho + 64, hj, mt * 128:(mt + 1) * 128],
                           qT_sb[ho:ho + 64, hj, :], start=True, stop=False)
                        for d in d_list:
                            mm(s2[:, i, :],
                               id_sb[:, h * ND + 2 * d : h * ND + 2 * d + 2, :],
                               mall[:, mt, d:d + 2, :],
                               start=False, stop=(d == d_list[-1]),
                               perf_mode=DR)
                    pT = pT_pool.tile([128, 2, NS], bf16, tag="pT")
                    nc.scalar.activation(pT[:], s2[:], AFT.Exp)
                    for i in range(2):
                        mt = 2 * pr + i
                        mm(o_ps[ih][0:65, :], v_sb[:, mt, h, :], pT[:, i, :],
                           start=(mt == 0), stop=(mt == MT - 1))
                if pr == 0 and pending_norm is not None:
                    # previous pass's normalization: the DVE reciprocal
                    # latency hides under this pass's first-pair PE work
                    pending_norm()
                    pending_norm = None
            pending_norm = make_norm(p, heads, o_ps, last=(p == 3))

        # ---- Phase C: output projection ----
        # last normalization's PE broadcasts go first (they park while the
        # g=0..2 pre-accumulation runs); only the g=3 matmuls and epilogues
        # sit in the drain tail
        pending_norm()
        psC = [psS.tile([128, 2, NS], f32, tag="psS", name=f"psC{a}")
               for a in range(2)]
        for jt in range(4):
            ps = psC[jt // 2][:, jt % 2, :]
            for g in range(3):
                mm(ps, wo_sb[:, g, jt * 128:(jt + 1) * 128],
                   OT_sb[:, g, :], start=(g == 0), stop=False)
        for jt in range(4):
            ps = psC[jt // 2][:, jt % 2, :]
            mm(ps, wo_sb[:, 3, jt * 128:(jt + 1) * 128],
               OT_sb[:, 3, :], start=False, stop=True)
            # split the bias epilogues across Act and DVE so the four
            # drains pipeline two-wide
            if jt % 2 == 0:
                nc.scalar.activation(outT_sb[:, jt, :], ps, AFT.Identity,
                                     bias=bo_sb[:, jt:jt + 1])
            else:
                nc.vector.tensor_scalar(outT_sb[:, jt, :], ps,
                                        bo_sb[:, jt:jt + 1], None, ALU.add)
            # alternate DMA queues so the four drains overlap
            eng = nc.sync if jt % 2 == 0 else nc.gpsimd
            eng.dma_start(outT[jt * 128:(jt + 1) * 128, :],
                          outT_sb[:, jt, :])


# every TPB instruction encoding in this walrus build tolerates only a
# single semaphore wait -- split extras regardless of opcode
_NO_SPLIT_TYPES = {"InstEventSemaphore"}


def _split_matmul_waits(nc, mybir):
    """Several engine instruction encodings tolerate only one semaphore
    wait; move extra waits onto standalone single-wait EventSemaphore
    instructions inserted right before them on the same engine queue."""
    import bass_rust

    n = 0
    for bb in nc.m.functions[0].blocks:
        insts = list(bb.instructions)
        out = []
        changed = False
        for i in insts:
            si = i.sync_info
            if (type(i).__name__ not in _NO_SPLIT_TYPES and si is not None
                    and len(si.on_wait) > 1):
                w = list(si.on_wait)
                for wx in w[:-1]:
                    ev = mybir.InstEventSemaphore(name=f"mmw_{n}_{i.name}",
                                                  ins=[], outs=[])
                    ev.engine = i.engine
                    ev.sync_info = bass_rust.SyncInfo(on_wait=[wx],
                                                      on_update=[])
                    out.append(ev)
                    n += 1
                si.on_wait = [w[-1]]
                changed = True
            out.append(i)
        if changed:
            bb.instructions = out


def _pick_dr3(absres, c):
    """Give the full 3-DoubleRow (exact-residual) mask path to the K3
    heads where dropping the r1/r2 residuals hurts most.  A coefficient
    error delta_r shifts the weight of class-r keys, whose share of the
    softmax mass scales as exp(-c_r), so impact ~ |delta_r|*exp(-c_r).
    The rest run the 2-DoubleRow variant (r0 still corrected via the
    duplicate M0 plane)."""
    k3 = _B("K3", 0)
    score = np.sum(absres[:, 1:] * np.exp(-c[:, 1:]), axis=1)
    order = np.argsort(score)[::-1]
    dr3 = [False] * H
    for h in order[:k3]:
        dr3[int(h)] = True
    return tuple(dr3)


def _host_prep(inputs):
    x_q = np.asarray(inputs["x_q"], np.float32)
    x_k = np.asarray(inputs["x_k"], np.float32)
    x_v = np.asarray(inputs["x_v"], np.float32)
    attn_mask = np.asarray(inputs["attn_mask"]).astype(bool)
    kpm = np.asarray(inputs["key_padding_mask"]).astype(bool)
    Wq = np.asarray(inputs["Wq"], np.float32)
    Wk = np.asarray(inputs["Wk"], np.float32)
    Wv = np.asarray(inputs["Wv"], np.float32)
    Wo = np.asarray(inputs["Wo"], np.float32)
    bq = np.asarray(inputs["bq"], np.float32)
    bv = np.asarray(inputs["bv"], np.float32)
    bo = np.asarray(inputs["bo"], np.float32)
    mw = np.asarray(inputs["mask_weight"], np.float64)

    # c[h,r] = softmax(mask_weight[h,:R]) * mask_weight[h,R]
    e = np.exp(mw[:, :R] - mw[:, :R].max(axis=1, keepdims=True))
    w = e / e.sum(axis=1, keepdims=True)
    c = (w * mw[:, R:R + 1]).astype(np.float64)          # [H, R]

    # split each coefficient into fp8 main + fp8 residual; DoubleRow d
    # covers plane pair (d, d+1) of the staged planes [M0, M1, M2, M0dup].
    # 3-DR heads apply all six products [c0, c1, r1, c2, r2, r0]; heads
    # whose r1/r2 residuals are negligible use only DRs 0 and 2 with
    # slots [c0, c1, -, -, c2, r0].
    fp8 = ml_dtypes.float8_e4m3
    cm = c.astype(fp8)                                    # main
    cr = (c - cm.astype(np.float64)).astype(fp8)          # residual
    cmf = cm.astype(np.float32)
    crf = cr.astype(np.float32)
    dr3 = _pick_dr3(np.abs(c - cm.astype(np.float64)), c)
    slots = np.zeros((H, ND), np.float32)
    for h in range(H):
        if dr3[h]:
            slots[h] = [cmf[h, 0], cmf[h, 1], crf[h, 1],
                        cmf[h, 2], crf[h, 2], crf[h, 0]]
        else:
            slots[h] = [cmf[h, 0], cmf[h, 1], 0.0,
                        0.0, cmf[h, 2], crf[h, 0]]

    id8 = np.zeros((H * ND, 128, 128), np.float32)
    eye = np.eye(128, dtype=np.float32)
    for h in range(H):
        for d in range(ND):
            id8[h * ND + d] = eye * (-slots[h, d])
    # partition-major so the DMA is one contiguous descriptor per row
    id8 = np.ascontiguousarray(
        id8.transpose(1, 0, 2)).reshape(128, H * ND * 128).astype(fp8)

    scale = np.float32(1.0 / np.sqrt(C))
    bq_s = (bq * scale).astype(np.float32)
    bo_p = (bv @ Wo + bo).astype(np.float32)

    bq2 = np.ascontiguousarray(bq_s.reshape(4, 128).T)
    bo2 = np.ascontiguousarray(bo_p.reshape(4, 128).T)

    def split8(a):
        # split a (in the 16x domain for weights) into fp8 main + residual
        hi = a.astype(fp8)
        lo = (a - hi.astype(np.float32)).astype(fp8)
        return hi, lo

    bf = ml_dtypes.bfloat16
    wq8, wqr = split8(np.ascontiguousarray(Wq) * np.float32(16.0))
    wk8, wkr = split8(np.ascontiguousarray(Wk) * np.float32(16.0))
    wv8, wvr = split8(np.ascontiguousarray(Wv) * np.float32(16.0))
    common = dict(wq8=wq8, wqr=wqr, wk8=wk8, wkr=wkr, wv8=wv8, wvr=wvr,
                  wo=Wo.astype(bf), id8=id8, bq2=bq2, bo2=bo2,
                  onesd=np.ones((1, 64), np.float32))

    in_maps = []
    for core in range(NCORES):
        b, ns = core // 4, core % 4
        n0 = ns * NS
        pad01 = (~kpm[b]).astype(np.float32) * np.float32(1.0 / 16.0)  # [N]
        pad2 = np.ascontiguousarray(pad01.reshape(MT, 128).T)
        pad8 = np.ascontiguousarray(
            np.repeat((pad2 != 0).astype(np.float32)[:, :, None], H, axis=2))
        m = dict(common)
        m["xtq8"], m["xtqr"] = split8(
            np.ascontiguousarray(x_q[n0:n0 + NS, b, :].T))
        xkT = np.ascontiguousarray(x_k[:, b, :].T)
        xvT = np.ascontiguousarray(x_v[:, b, :].T)
        m["xtk8"], m["xtkr"] = split8(xkT)
        m["xtv8"], m["xtvr"] = split8(xvT)
        m3 = attn_mask[b, :, n0:n0 + NS, :].transpose(2, 0, 1)  # [m, R, n]
        m4 = np.concatenate([m3, m3[:, 0:1]], axis=1)           # [m, 4, n]
        m["masksT"] = np.ascontiguousarray(m4).astype(np.float32).astype(fp8)
        m["pad"] = pad2
        m["pad8"] = pad8
        in_maps.append(m)
    return in_maps, dr3


def kernel(**inputs) -> np.ndarray:
    from concourse.bass_utils import run_bass_kernel_spmd

    in_maps, dr3 = _host_prep(inputs)
    if _cache.get("dr3") != dr3:
        _cache["nc"] = _build_program(dr3=dr3)
        _cache["dr3"] = dr3
    nc = _cache["nc"]

    res = run_bass_kernel_spmd(nc, in_maps, list(range(NCORES)))

    out = np.empty((N, B, D), np.float32)
    for core in range(NCORES):
        b, ns = core // 4, core % 4
        n0 = ns * NS
        out[n0:n0 + NS, b, :] = res.results[core]["outT"].astype(np.float32).T
    return out
